# revision 2
# baseline (speedup 1.0000x reference)
"""Llama GQA attention layer (B=1, S=2048, D=4096, H=32, KVH=8, DH=128) on 8 trn2 cores.

Sharding: tensor-parallel over heads. Core c owns Q heads [4c, 4c+4) and KV head c:
  Wq[:, c*512:(c+1)*512], Wk/Wv[:, c*128:(c+1)*128], Wo rows [c*512:(c+1)*512].

Host<->device traffic is the wall-clock bottleneck (axon-tunneled PJRT moves
~60-80MB/s with ~0.2s fixed cost per transfer), so the I/O contract is built
around moving as few bytes in as few tensors as possible:
  - ONE bf16 input blob per core [128, NB]: Wq/Wk/Wv/Wo shards (tiled layouts),
    cos^T/sin^T, causal 0/1 mask tiles, identity - everything the core needs.
  - x^T is sharded by d across cores ("gxt" [512, 2048] per core) and
    AllGather'd on device into a shared DRAM buffer (16.8MB gathered vs 134MB
    replicated upload).
  - The row-parallel Wo partial sums are ReduceScatter'd on device (fp32, CCE
    adds), each core returns only its [256, 4096] slice as bf16 - 16.8MB total
    download instead of 268MB fp32 partials + host-side sum.

Kernel compute (per core), unchanged from the tuned single-blob version:
  - X^T streamed from the gathered buffer; Q^T/K^T/V^T [dh, s] via PSUM
    accumulation over 32 d-tiles; RoPE on PSUM evacuation (DVE).
  - V^T transposed to V natural via PE-transpose.
  - Attention with scores transposed: S^T[k, q] tiles [128, 512]; softmax sums
    over keys via ones-vector matmuls; exp on ACT (scores are O(10), safe);
    causal sparsity by skipping fully-masked key tiles; diagonal tiles masked
    multiplicatively with 4 static 0/1 tiles.
  - Output projection accumulates over the 4 head-blocks into a DRAM partial,
    then ReduceScatter + bf16 cast out.
"""

import numpy as np

import concourse.bass as bass
import concourse.bacc as bacc
import concourse.mybir as mybir
import concourse.tile as tile
from concourse.bass_utils import run_bass_kernel_spmd

S = 2048
D = 4096
H = 32
KVH = 8
DH = 128
NCORES = 8
HPC = H // NCORES            # 4 query heads per core
QC = HPC * DH                # 512 projection cols per core
SCALE = float(DH) ** -0.5
NT_D = D // 128              # 32 contraction tiles
NCH = S // 512               # 4 sequence chunks
SROW = S // NCORES           # 256 output rows per core after reduce-scatter
FP32 = mybir.dt.float32
FP32R = mybir.dt.float32r
BF16 = mybir.dt.bfloat16
AF = mybir.ActivationFunctionType
GROUPS = [list(range(NCORES))]

import os as _os
MMDT = {"bf16": BF16, "fp32r": FP32R}[_os.environ.get("KERNEL_MM_DTYPE", "bf16")]

# blob column offsets (bf16 [128, NB])
CB_WQ = 0
CB_WK = CB_WQ + NT_D * QC            # 16384
CB_WV = CB_WK + NT_D * DH            # 20480
CB_WO = CB_WV + NT_D * DH            # 24576
CB_CS = CB_WO + (D // 512) * HPC * 512   # 40960
CB_SN = CB_CS + S                    # 43008
CB_MSK = CB_SN + S                   # 45056
CB_ID = CB_MSK + S                   # 47104
NB = CB_ID + 128                     # 47232


def _np_mmdt():
    import ml_dtypes
    return {BF16: ml_dtypes.bfloat16, FP32R: np.float32}[MMDT]


def _emit(nc, tc, io, mode, phases="ABC"):
    """mode: 'causal' (sparse, static diag masks), 'dense' (all tiles, no mask),
    'masked' (all tiles, additive mask streamed from DRAM)."""
    from contextlib import ExitStack

    blob_d, gxt_d, mskf_d, out_d = io

    with ExitStack() as top:
        ep = top.enter_context  # persistent pools

        # ---------- persistent DRAM (whole kernel) ----------
        dram = ep(tc.tile_pool(name="dram", bufs=1, space="DRAM"))
        gin = dram.tile([512, S], MMDT, name="gin")
        gx = dram.tile([D, S], MMDT, name="gx", addr_space="Shared")
        po = dram.tile([S, D], FP32, name="po")
        rs = dram.tile([SROW, D], FP32, name="rs")

        # x^T all-gather: d-tiles [4c, 4c+4) from each core -> full [D, S]
        nc.gpsimd.dma_start(gin[:], gxt_d[:])
        nc.gpsimd.collective_compute(
            "AllGather", mybir.AluOpType.bypass, replica_groups=GROUPS,
            ins=[gin.opt()], outs=[gx.opt()])

        # ---------- persistent SBUF (whole kernel) ----------
        pers = ep(tc.tile_pool(name="pers", bufs=1))
        qt = pers.tile([128, HPC * S], MMDT, name="qt")        # Q^T, head h at [:, h*S:(h+1)*S]
        kt = pers.tile([128, S], MMDT, name="kt")              # K^T
        vn = pers.tile([128, S], MMDT, name="vn")              # V natural, tile t at [:, 128t:128t+128]
        at = pers.tile([128, HPC * S], MMDT, name="at")        # attn^T
        ones_c = pers.tile([128, 1], MMDT, name="ones_c")
        ones_r = pers.tile([1, 128], FP32, name="ones_r")
        msk_sb = pers.tile([128, 4 * 512], MMDT, name="msk_sb")

        # ================= Phase A: projections =================
        with ExitStack() as pa:
            e = pa.enter_context
            wpool = e(tc.tile_pool(name="wpool", bufs=1))
            id_sb = wpool.tile([128, 128], MMDT, name="id_sb")
            nc.sync.dma_start(id_sb[:], blob_d[:, CB_ID:CB_ID + 128])
            cs_b = wpool.tile([128, S], MMDT, name="cs_b")
            sn_b = wpool.tile([128, S], MMDT, name="sn_b")
            cs_sb = wpool.tile([128, S], FP32, name="cs_sb")
            sn_sb = wpool.tile([128, S], FP32, name="sn_sb")
            xpool = e(tc.tile_pool(name="xpool", bufs=3))
            tpool = e(tc.tile_pool(name="tpool", bufs=2))
            psum = e(tc.tile_pool(name="psumA", bufs=1, space=bass.MemorySpace.PSUM))

            # startup order: the tiles gating the first matmuls go first
            wq_t2 = [wpool.tile([128, 2 * QC], MMDT, name=f"wq2_{i}")
                     for i in range(NT_D // 2)]
            wk_t8 = [wpool.tile([128, 8 * DH], MMDT, name=f"wk8_{i}")
                     for i in range(NT_D // 8)]
            wv_t8 = [wpool.tile([128, 8 * DH], MMDT, name=f"wv8_{i}")
                     for i in range(NT_D // 8)]
            nc.sync.dma_start(wq_t2[0][:], blob_d[:, CB_WQ:CB_WQ + 2 * QC])
            nc.sync.dma_start(wk_t8[0][:], blob_d[:, CB_WK:CB_WK + 8 * DH])
            nc.sync.dma_start(wv_t8[0][:], blob_d[:, CB_WV:CB_WV + 8 * DH])
            nc.vector.memset(ones_c[:], 1.0)
            nc.vector.memset(ones_r[:], 1.0)
            if mode == "causal":
                nc.sync.dma_start(msk_sb[:], blob_d[:, CB_MSK:CB_MSK + 4 * 512])
            for i in range(1, NT_D // 2):
                nc.sync.dma_start(wq_t2[i][:],
                                  blob_d[:, CB_WQ + i * 2 * QC:CB_WQ + (i + 1) * 2 * QC])
            for i in range(1, NT_D // 8):
                nc.sync.dma_start(wk_t8[i][:],
                                  blob_d[:, CB_WK + i * 8 * DH:CB_WK + (i + 1) * 8 * DH])
                nc.sync.dma_start(wv_t8[i][:],
                                  blob_d[:, CB_WV + i * 8 * DH:CB_WV + (i + 1) * 8 * DH])
            nc.sync.dma_start(cs_b[:], blob_d[:, CB_CS:CB_CS + S])
            nc.sync.dma_start(sn_b[:], blob_d[:, CB_SN:CB_SN + S])
            nc.scalar.copy(cs_sb[:], cs_b[:])
            nc.scalar.copy(sn_sb[:], sn_b[:])

            def wq_ap(dt_, h):
                return wq_t2[dt_ // 2][:, (dt_ % 2) * QC + h * 128:
                                       (dt_ % 2) * QC + (h + 1) * 128]

            def wk_ap(dt_):
                return wk_t8[dt_ // 8][:, (dt_ % 8) * DH:(dt_ % 8 + 1) * DH]

            def wv_ap(dt_):
                return wv_t8[dt_ // 8][:, (dt_ % 8) * DH:(dt_ % 8 + 1) * DH]

            def rope_evac(src_ps, dest, ci):
                cs = cs_sb[:, ci * 512:(ci + 1) * 512]
                sn = sn_sb[:, ci * 512:(ci + 1) * 512]
                t1 = tpool.tile([128, 512], FP32, tag="t1", bufs=2)
                t2 = tpool.tile([128, 512], FP32, tag="t2", bufs=2)
                nc.vector.tensor_mul(t1[:], src_ps[:], cs)
                nc.vector.tensor_mul(t2[0:64, :], src_ps[64:128, :], sn[0:64, :])
                nc.vector.tensor_mul(t2[64:128, :], src_ps[0:64, :], sn[64:128, :])
                nc.vector.tensor_sub(dest[0:64, :], t1[0:64, :], t2[0:64, :])
                nc.vector.tensor_add(dest[64:128, :], t1[64:128, :], t2[64:128, :])

            for ci in range(NCH):
                acc = [psum.tile([128, 512], FP32, tag="acc", bufs=6,
                                 name=f"acc{ci}_{b}") for b in range(6)]
                for i in range(NT_D // 2):
                    xt_t = xpool.tile([128, 1024], MMDT, tag="xt", bufs=4)
                    nc.sync.dma_start(
                        xt_t[:, 0:512],
                        gx[2 * i * 128:(2 * i + 1) * 128, ci * 512:(ci + 1) * 512])
                    nc.sync.dma_start(
                        xt_t[:, 512:1024],
                        gx[(2 * i + 1) * 128:(2 * i + 2) * 128, ci * 512:(ci + 1) * 512])
                    for half in range(2):
                        dt_ = 2 * i + half
                        st = dt_ == 0
                        sp = dt_ == NT_D - 1
                        rhs = xt_t[:, half * 512:(half + 1) * 512]
                        for h in range(HPC):
                            nc.tensor.matmul(acc[h][:], wq_ap(dt_, h), rhs,
                                             start=st, stop=sp)
                        nc.tensor.matmul(acc[4][:], wk_ap(dt_), rhs,
                                         start=st, stop=sp)
                        nc.tensor.matmul(acc[5][:], wv_ap(dt_), rhs,
                                         start=st, stop=sp)
                for h in range(HPC):
                    rope_evac(acc[h], qt[:, h * S + ci * 512:h * S + (ci + 1) * 512], ci)
                rope_evac(acc[4], kt[:, ci * 512:(ci + 1) * 512], ci)
                # V: plain evac then PE-transpose each 128 block to natural layout
                vt_t = tpool.tile([128, 512], MMDT, tag="vt", bufs=2)
                nc.scalar.copy(vt_t[:], acc[5][:])
                for i in range(4):
                    ps_tr = psum.tile([128, 128], MMDT, tag="tr", bufs=2,
                                      name=f"tr{ci}_{i}")
                    nc.tensor.transpose(ps_tr[:], vt_t[:, i * 128:(i + 1) * 128], id_sb[:])
                    s0 = (ci * 4 + i) * 128
                    nc.vector.tensor_copy(vn[:, s0:s0 + 128], ps_tr[:])

        if "B" not in phases:
            return

        # ================= Phase B: attention =================
        with ExitStack() as pb:
            e = pb.enter_context
            ppool = e(tc.tile_pool(name="ppool", bufs=4))
            npool = e(tc.tile_pool(name="npool", bufs=2))
            mpool = e(tc.tile_pool(name="mpool", bufs=4))
            psum = e(tc.tile_pool(name="psumB", bufs=1, space=bass.MemorySpace.PSUM))

            for ci in range(NCH):
                n_sk = 4 * (ci + 1) if mode == "causal" else S // 128
                for h in range(HPC):
                    ps_pv = psum.tile([128, 512], FP32, tag="pv", bufs=2,
                                      name=f"pv{ci}_{h}")
                    ps_sm = psum.tile([1, 512], FP32, tag="sm", bufs=2,
                                      name=f"sm{ci}_{h}")
                    qs = qt[:, h * S + ci * 512:h * S + (ci + 1) * 512]
                    for sk in range(n_sk):
                        ps_sc = psum.tile([128, 512], FP32, tag="sc", bufs=2,
                                          name=f"sc{ci}_{h}_{sk}")
                        nc.tensor.matmul(ps_sc[:], kt[:, sk * 128:(sk + 1) * 128],
                                         qs, start=True, stop=True)
                        p = ppool.tile([128, 512], MMDT, tag="p", bufs=4)
                        if mode == "masked":
                            mt = mpool.tile([128, 512], FP32, tag="mt", bufs=4)
                            nc.sync.dma_start(
                                mt[:], mskf_d[sk * 128:(sk + 1) * 128,
                                              ci * 512:(ci + 1) * 512])
                            nc.vector.tensor_scalar_mul(p[:], ps_sc[:], SCALE)
                            nc.vector.tensor_add(p[:], p[:], mt[:])
                            nc.scalar.activation(p[:], p[:], AF.Exp)
                        else:
                            nc.scalar.activation(p[:], ps_sc[:], AF.Exp, scale=SCALE)
                            if mode == "causal" and sk >= 4 * ci:
                                j = sk - 4 * ci
                                nc.vector.tensor_mul(
                                    p[:], p[:], msk_sb[:, j * 512:(j + 1) * 512])
                        st = sk == 0
                        sp = sk == n_sk - 1
                        nc.tensor.matmul(ps_pv[:], vn[:, sk * 128:(sk + 1) * 128],
                                         p[:], start=st, stop=sp)
                        nc.tensor.matmul(ps_sm[:], ones_c[:], p[:],
                                         start=st, stop=sp)
                    # normalize: 1/sums broadcast over partitions via K=1 matmul
                    rc = npool.tile([1, 512], FP32, tag="rc", bufs=2)
                    rrs = npool.tile([1, 512], FP32, tag="rs", bufs=2)
                    nc.vector.reciprocal_approx_accurate(rc[:], ps_sm[:], rrs[:])
                    ps_bc = psum.tile([128, 512], FP32, tag="bc", bufs=2,
                                      name=f"bc{ci}_{h}")
                    nc.tensor.matmul(ps_bc[:], ones_r[:], rc[:], start=True, stop=True)
                    rb = npool.tile([128, 512], FP32, tag="rb", bufs=2)
                    nc.scalar.copy(rb[:], ps_bc[:])
                    nc.vector.tensor_mul(at[:, h * S + ci * 512:h * S + (ci + 1) * 512],
                                         ps_pv[:], rb[:])

        if "C" not in phases:
            return
        # ================= Phase C: output projection -> DRAM partial =================
        with ExitStack() as pc:
            e = pc.enter_context
            wopool = e(tc.tile_pool(name="wopool", bufs=8))
            opool = e(tc.tile_pool(name="opool", bufs=4))
            psum = e(tc.tile_pool(name="psumC", bufs=1, space=bass.MemorySpace.PSUM))
            for op_ in range(D // 1024):
                wt = []
                for odh in range(2):
                    od = 2 * op_ + odh
                    w = wopool.tile([128, HPC * 512], MMDT, tag="wo", bufs=4)
                    nc.sync.dma_start(w[:], blob_d[:, CB_WO + od * HPC * 512:
                                                   CB_WO + (od + 1) * HPC * 512])
                    wt.append(w)
                for sb in range(S // 128):
                    ob = opool.tile([128, 1024], FP32, tag="ob", bufs=4)
                    for odh in range(2):
                        ps_o = psum.tile([128, 512], FP32, tag="oo", bufs=4,
                                         name=f"oo{op_}_{sb}_{odh}")
                        for h in range(HPC):
                            nc.tensor.matmul(
                                ps_o[:],
                                at[:, h * S + sb * 128:h * S + (sb + 1) * 128],
                                wt[odh][:, h * 512:(h + 1) * 512],
                                start=(h == 0), stop=(h == HPC - 1))
                        nc.vector.tensor_copy(ob[:, odh * 512:(odh + 1) * 512],
                                              ps_o[:])
                    nc.sync.dma_start(po[sb * 128:(sb + 1) * 128,
                                         op_ * 1024:(op_ + 1) * 1024], ob[:])

        # ====== reduce-scatter partials across cores, emit bf16 shard ======
        with ExitStack() as pd_:
            e = pd_.enter_context
            spool = e(tc.tile_pool(name="spool", bufs=2))
            nc.gpsimd.collective_compute(
                "ReduceScatter", mybir.AluOpType.add, replica_groups=GROUPS,
                ins=[po.opt()], outs=[rs.opt()])
            for i in range(SROW // 128):
                t = spool.tile([128, D], FP32, tag="t", bufs=2)
                nc.sync.dma_start(t[:], rs[i * 128:(i + 1) * 128, :])
                tb = spool.tile([128, D], MMDT, tag="tb", bufs=2)
                nc.scalar.copy(tb[:], t[:])
                nc.sync.dma_start(out_d[i * 128:(i + 1) * 128, :], tb[:])


def build(mode="causal", phases="ABC"):
    nc = bacc.Bacc("TRN2", target_bir_lowering=False, debug=False,
                   num_devices=NCORES)
    blob_d = nc.dram_tensor("blob", [128, NB], MMDT, kind="ExternalInput").ap()
    gxt_d = nc.dram_tensor("gxt", [512, S], MMDT, kind="ExternalInput").ap()
    mskf_d = None
    if mode == "masked":
        mskf_d = nc.dram_tensor("msk", [S, S], FP32, kind="ExternalInput").ap()
    out_d = nc.dram_tensor("out", [SROW, D], MMDT, kind="ExternalOutput").ap()
    io = (blob_d, gxt_d, mskf_d, out_d)
    with tile.TileContext(nc) as tc:
        _emit(nc, tc, io, mode, phases)
    nc.compile()
    return nc


_CACHE = {}
RUN_KWARGS = {}   # extra kwargs for run_bass_kernel_spmd (e.g. trace=True)
LAST = None       # last BassKernelResults (for exec_time_ns inspection)


def _causal_ref_mask():
    neg = np.finfo(np.float32).min
    m = np.where(np.tril(np.ones((S, S), dtype=bool)), 0.0, neg)
    return m.astype(np.float32)


def _tile_rows(w):
    # [T*128, C] -> [128, T*C] with d-tile blocks along free dim
    t = w.shape[0] // 128
    return np.ascontiguousarray(
        w.reshape(t, 128, w.shape[1]).transpose(1, 0, 2).reshape(128, -1))


def _tile_wo(w):
    # [512, D] -> [128, (od, h) blocks]: block (h, od) at [p, od*2048 + h*512]
    return np.ascontiguousarray(
        w.reshape(HPC, 128, D // 512, 512).transpose(1, 2, 0, 3).reshape(128, -1))


def make_in_maps(hidden_states, cos, sin, attention_mask, Wq, Wk, Wv, Wo, mode):
    mdt = _np_mmdt()
    xT = np.ascontiguousarray(
        np.asarray(hidden_states).reshape(S, D).T).astype(mdt)   # [D, S]
    cosT = np.ascontiguousarray(np.asarray(cos).T).astype(mdt)   # [128, S]
    sinT = np.ascontiguousarray(np.asarray(sin).T).astype(mdt)
    ident = np.eye(128, dtype=mdt)
    Wqb = np.asarray(Wq).astype(mdt)
    Wkb = np.asarray(Wk).astype(mdt)
    Wvb = np.asarray(Wv).astype(mdt)
    Wob = np.asarray(Wo).astype(mdt)
    if mode == "causal":
        # 4 diagonal 0/1 tiles: tile j valid where 128*j + k <= q  (k:[128], q:[512])
        j = np.arange(4)[:, None, None]
        k = np.arange(128)[None, :, None]
        q = np.arange(512)[None, None, :]
        msk = np.ascontiguousarray((128 * j + k <= q).astype(mdt)
                                   .transpose(1, 0, 2).reshape(128, 2048))
    else:
        msk = np.zeros((128, 2048), dtype=mdt)
    mskf = None
    if mode == "masked":
        mskf = np.ascontiguousarray(
            np.asarray(attention_mask).reshape(S, S).T).astype(np.float32)
    in_maps = []
    for c in range(NCORES):
        blob = np.empty((128, NB), dtype=mdt)
        blob[:, CB_WQ:CB_WK] = _tile_rows(Wqb[:, c * QC:(c + 1) * QC])
        blob[:, CB_WK:CB_WV] = _tile_rows(Wkb[:, c * DH:(c + 1) * DH])
        blob[:, CB_WV:CB_WO] = _tile_rows(Wvb[:, c * DH:(c + 1) * DH])
        blob[:, CB_WO:CB_CS] = _tile_wo(Wob[c * QC:(c + 1) * QC, :])
        blob[:, CB_CS:CB_SN] = cosT
        blob[:, CB_SN:CB_MSK] = sinT
        blob[:, CB_MSK:CB_ID] = msk
        blob[:, CB_ID:NB] = ident
        m = {"blob": blob, "gxt": xT[c * 512:(c + 1) * 512]}
        if mode == "masked":
            m["msk"] = mskf
        in_maps.append(m)
    return in_maps


def pick_mode(attention_mask):
    am = np.asarray(attention_mask).reshape(S, S)
    if not np.any(am):
        return "dense"
    if np.array_equal(am, _causal_ref_mask()):
        return "causal"
    return "masked"


def kernel(hidden_states, cos, sin, attention_mask, Wq, Wk, Wv, Wo, **kwargs):
    mode = pick_mode(attention_mask)
    ck = (mode, str(MMDT))
    if ck not in _CACHE:
        _CACHE[ck] = build(mode)
    nc = _CACHE[ck]
    in_maps = make_in_maps(hidden_states, cos, sin, attention_mask,
                           Wq, Wk, Wv, Wo, mode)
    res = run_bass_kernel_spmd(nc, in_maps, core_ids=list(range(NCORES)),
                               **RUN_KWARGS)
    global LAST
    LAST = res
    out = np.concatenate([res.results[c]["out"] for c in range(NCORES)], axis=0)
    return out.astype(np.float32).reshape(1, S, D)


# revision 15
# speedup vs baseline: 1.0121x; 1.0121x over previous
"""Llama GQA attention layer (B=1, S=2048, D=4096, H=32, KVH=8, DH=128) on 8 trn2 cores.

Sharding: tensor-parallel over heads. Core c owns Q heads [4c, 4c+4) and KV head c:
  Wq[:, c*512:(c+1)*512], Wk/Wv[:, c*128:(c+1)*128], Wo rows [c*512:(c+1)*512].

Host<->device traffic is the wall-clock bottleneck (axon-tunneled PJRT moves
~80MB/s with ~0.1-0.2s fixed cost per transfer / per fetched shard), so the
I/O contract is built around moving as few bytes in as few tensors as possible:
  - ONE int8 input blob per core [128, NBYTES]: weight shards (a configurable
    subset quantized to int8 with per-d-row scales, the rest fp16), the core's
    x^T d-shard, cos^T/sin^T, causal 0/1 mask tiles, identity, Wo scales row.
    fp16 segments are byte-packed and read back with AP.bitcast.
  - All quantization scale corrections are folded into host-side constants:
    x^T is pre-scaled by the shared qkv row scale, unquantized q/k/v weights
    are pre-divided by it, the exp() scale constant absorbs P factors, and the
    Wo scales ride the softmax-normalization outer-product matmul (the scale
    row replaces the ones row - zero extra device instructions).
  - x^T is sharded by d across cores and AllGather'd on device (2.1MB/core
    uploaded instead of 16.8MB replicated).
  - The row-parallel Wo partial sums are ReduceScatter'd on device (fp32 CCE
    adds); each core returns only its [256, 4096] slice as fp16 - 16.8MB total
    download instead of 268MB fp32 partials + host-side sum.

Kernel compute (per core):
  - X^T streamed from the gathered buffer; Q^T/K^T/V^T [dh, s] via PSUM
    accumulation over 32 d-tiles; RoPE on PSUM evacuation (DVE).
  - V^T transposed to V natural via PE-transpose.
  - Attention with scores transposed: S^T[k, q] tiles [128, 512]; softmax sums
    over keys via ones-vector matmuls; exp on ACT; causal sparsity by skipping
    fully-masked key tiles; diagonal tiles masked multiplicatively.
  - Output projection accumulates over the 4 head-blocks into a DRAM partial,
    then ReduceScatter + fp16 cast out.
"""

import os as _os

import numpy as np

import concourse.bass as bass
import concourse.bacc as bacc
import concourse.mybir as mybir
import concourse.tile as tile
from concourse.bass_utils import run_bass_kernel_spmd

S = 2048
D = 4096
H = 32
KVH = 8
DH = 128
NCORES = 8
HPC = H // NCORES            # 4 query heads per core
QC = HPC * DH                # 512 projection cols per core
SCALE = float(DH) ** -0.5
NT_D = D // 128              # 32 contraction tiles
NCH = S // 512               # 4 sequence chunks
SROW = S // NCORES           # 256 output rows per core after reduce-scatter
FP32 = mybir.dt.float32
BF16 = mybir.dt.bfloat16
FP16 = mybir.dt.float16
INT8 = mybir.dt.int8
AF = mybir.ActivationFunctionType
GROUPS = [list(range(NCORES))]

MMDT = {"fp16": FP16, "bf16": BF16}[_os.environ.get("KERNEL_MM_DTYPE", "fp16")]
_KQ8 = _os.environ.get("KERNEL_Q8", "vo")     # which of Wq/Wk/Wv/Wo are int8
Q8, K8, V8, O8 = ("q" in _KQ8), ("k" in _KQ8), ("v" in _KQ8), ("o" in _KQ8)
PSC = 1024.0                  # power-of-2 renorm to keep fp16 segments normal
CSC = 1024.0                  # at-tile upscale, removed on the ob evacuation
EXPB = 5.0                    # exp(score - EXPB): keeps p under fp16 max 65504
                              # (scores reach ~11.3; shift cancels in the sum
                              # normalization exactly)

# blob byte offsets (int8 [128, NBYTES]; 16-bit segments byte-packed)
def _seg(prev, nbytes):
    return prev, prev + nbytes

OB_WQ, _e = _seg(0, NT_D * QC * (1 if Q8 else 2))
OB_WK, _e = _seg(_e, NT_D * DH * (1 if K8 else 2))
OB_WV, _e = _seg(_e, NT_D * DH * (1 if V8 else 2))
OB_WO, _e = _seg(_e, (D // 512) * HPC * 512 * (1 if O8 else 2))
OB_X, _e = _seg(_e, 2 * 4 * S)
OB_CS, _e = _seg(_e, 2 * S)
OB_SN, _e = _seg(_e, 2 * S)
OB_MSK, _e = _seg(_e, 2 * S)
OB_ID, _e = _seg(_e, 2 * 128)
OB_SWO, NBYTES = _seg(_e, 2 * 512)


def _np_mmdt():
    import ml_dtypes
    return {FP16: np.float16, BF16: ml_dtypes.bfloat16}[MMDT]


def _emit(nc, tc, io, mode, phases="ABC"):
    """mode: 'causal' (sparse, static diag masks), 'dense' (all tiles, no mask),
    'masked' (all tiles, additive mask streamed from DRAM)."""
    from contextlib import ExitStack

    blob_d, mskf_d, out_d = io
    dbg = {}
    if DBG:
        for nm, cols in [("dqt", HPC * S), ("dkt", S), ("dvn", S), ("dat", HPC * S)]:
            dbg[nm] = nc.dram_tensor(nm, [128, cols], MMDT, kind="ExternalOutput").ap()
    n_p8 = (1 if Q8 else 0) + (1 if K8 else 0)
    es = SCALE / (PSC ** n_p8)

    with ExitStack() as top:
        ep = top.enter_context  # persistent pools

        # ---------- persistent DRAM (whole kernel) ----------
        dram = ep(tc.tile_pool(name="dram", bufs=1, space="DRAM"))
        gin = dram.tile([512, S], MMDT, name="gin")
        gx = dram.tile([D, S], MMDT, name="gx", addr_space="Shared")
        po = dram.tile([S, D], FP32, name="po")
        rs = dram.tile([SROW, D], FP32, name="rs")

        # x^T all-gather: d-tiles [4c, 4c+4) from each core -> full [D, S]
        nc.gpsimd.dma_start(gin[:], blob_d[:, OB_X:OB_X + 2 * 4 * S].bitcast(MMDT))
        nc.gpsimd.collective_compute(
            "AllGather", mybir.AluOpType.bypass, replica_groups=GROUPS,
            ins=[gin.opt()], outs=[gx.opt()])

        # ---------- persistent SBUF (whole kernel) ----------
        pers = ep(tc.tile_pool(name="pers", bufs=1))
        qt = pers.tile([128, HPC * S], MMDT, name="qt")        # Q^T, head h at [:, h*S:(h+1)*S]
        kt = pers.tile([128, S], MMDT, name="kt")              # K^T
        vn = pers.tile([128, S], MMDT, name="vn")              # V natural, tile t at [:, 128t:128t+128]
        at = pers.tile([128, HPC * S], MMDT, name="at")        # attn^T (pre-scaled, see swo)
        ones_c = pers.tile([128, 1], MMDT, name="ones_c")
        expb_c = pers.tile([128, 1], FP32, name="expb_c")      # -EXPB bias for exp
        swo_f = pers.tile([1, 512], FP32, name="swo_f")        # Wo row scales
        msk_sb = pers.tile([128, 4 * 512], MMDT, name="msk_sb")

        # ================= Phase A: projections =================
        with ExitStack() as pa:
            e = pa.enter_context
            wpool = e(tc.tile_pool(name="wpool", bufs=1))
            w8pool = e(tc.tile_pool(name="w8pool", bufs=3))
            # identity in fp32: the PE transpose path is exact in fp32 and
            # requires out/in/identity dtypes to line up (fp16 transpose is broken)
            id_b = wpool.tile([128, 128], MMDT, name="id_b")
            id_sb = wpool.tile([128, 128], FP32, name="id_sb")
            nc.sync.dma_start(id_b[:], blob_d[:, OB_ID:OB_ID + 256].bitcast(MMDT))
            nc.scalar.copy(id_sb[:], id_b[:])
            cs_b = wpool.tile([128, S], MMDT, name="cs_b")
            sn_b = wpool.tile([128, S], MMDT, name="sn_b")
            cs_sb = wpool.tile([128, S], FP32, name="cs_sb")
            sn_sb = wpool.tile([128, S], FP32, name="sn_sb")
            swo_b = wpool.tile([1, 512], MMDT, name="swo_b")
            xpool = e(tc.tile_pool(name="xpool", bufs=3))
            tpool = e(tc.tile_pool(name="tpool", bufs=2))
            psum = e(tc.tile_pool(name="psumA", bufs=1, space=bass.MemorySpace.PSUM))

            wq_t2 = [wpool.tile([128, 2 * QC], MMDT, name=f"wq2_{i}")
                     for i in range(NT_D // 2)]
            wk_t8 = [wpool.tile([128, 8 * DH], MMDT, name=f"wk8_{i}")
                     for i in range(NT_D // 8)]
            wv_t8 = [wpool.tile([128, 8 * DH], MMDT, name=f"wv8_{i}")
                     for i in range(NT_D // 8)]

            def wload(dst, q8, off, ncols):
                # int8 segment: DMA + dtype-convert copy; 16-bit: direct DMA
                if q8:
                    t8 = w8pool.tile([128, ncols], INT8, tag="w8", bufs=3)
                    nc.sync.dma_start(t8[:], blob_d[:, off:off + ncols])
                    nc.scalar.copy(dst[:], t8[:])
                else:
                    nc.sync.dma_start(
                        dst[:], blob_d[:, off:off + 2 * ncols].bitcast(MMDT))

            def wq_off(i):
                return OB_WQ + i * 2 * QC * (1 if Q8 else 2)

            def wk_off(i):
                return OB_WK + i * 8 * DH * (1 if K8 else 2)

            def wv_off(i):
                return OB_WV + i * 8 * DH * (1 if V8 else 2)

            # startup order: the tiles gating the first matmuls go first
            wload(wq_t2[0], Q8, wq_off(0), 2 * QC)
            wload(wk_t8[0], K8, wk_off(0), 8 * DH)
            wload(wv_t8[0], V8, wv_off(0), 8 * DH)
            nc.vector.memset(ones_c[:], 1.0)
            nc.vector.memset(expb_c[:], -EXPB)
            if mode == "causal":
                nc.sync.dma_start(msk_sb[:],
                                  blob_d[:, OB_MSK:OB_MSK + 2 * 4 * 512].bitcast(MMDT))
            for i in range(1, NT_D // 2):
                wload(wq_t2[i], Q8, wq_off(i), 2 * QC)
            for i in range(1, NT_D // 8):
                wload(wk_t8[i], K8, wk_off(i), 8 * DH)
                wload(wv_t8[i], V8, wv_off(i), 8 * DH)
            nc.sync.dma_start(cs_b[:], blob_d[:, OB_CS:OB_CS + 2 * S].bitcast(MMDT))
            nc.sync.dma_start(sn_b[:], blob_d[:, OB_SN:OB_SN + 2 * S].bitcast(MMDT))
            nc.sync.dma_start(swo_b[:],
                              blob_d[0:1, OB_SWO:OB_SWO + 2 * 512].bitcast(MMDT))
            nc.scalar.copy(cs_sb[:], cs_b[:])
            nc.scalar.copy(sn_sb[:], sn_b[:])
            nc.scalar.copy(swo_f[:], swo_b[:])

            def wq_ap(dt_, h):
                return wq_t2[dt_ // 2][:, (dt_ % 2) * QC + h * 128:
                                       (dt_ % 2) * QC + (h + 1) * 128]

            def wk_ap(dt_):
                return wk_t8[dt_ // 8][:, (dt_ % 8) * DH:(dt_ % 8 + 1) * DH]

            def wv_ap(dt_):
                return wv_t8[dt_ // 8][:, (dt_ % 8) * DH:(dt_ % 8 + 1) * DH]

            def rope_evac(src_ps, dest, ci):
                cs = cs_sb[:, ci * 512:(ci + 1) * 512]
                sn = sn_sb[:, ci * 512:(ci + 1) * 512]
                t1 = tpool.tile([128, 512], FP32, tag="t1", bufs=2)
                t2 = tpool.tile([128, 512], FP32, tag="t2", bufs=2)
                nc.vector.tensor_mul(t1[:], src_ps[:], cs)
                nc.vector.tensor_mul(t2[0:64, :], src_ps[64:128, :], sn[0:64, :])
                nc.vector.tensor_mul(t2[64:128, :], src_ps[0:64, :], sn[64:128, :])
                nc.vector.tensor_sub(dest[0:64, :], t1[0:64, :], t2[0:64, :])
                nc.vector.tensor_add(dest[64:128, :], t1[64:128, :], t2[64:128, :])

            for ci in range(NCH):
                acc = [psum.tile([128, 512], FP32, tag="acc", bufs=6,
                                 name=f"acc{ci}_{b}") for b in range(6)]
                for i in range(NT_D // 2):
                    xt_t = xpool.tile([128, 1024], MMDT, tag="xt", bufs=4)
                    nc.sync.dma_start(
                        xt_t[:, 0:512],
                        gx[2 * i * 128:(2 * i + 1) * 128, ci * 512:(ci + 1) * 512])
                    nc.sync.dma_start(
                        xt_t[:, 512:1024],
                        gx[(2 * i + 1) * 128:(2 * i + 2) * 128, ci * 512:(ci + 1) * 512])
                    for half in range(2):
                        dt_ = 2 * i + half
                        st = dt_ == 0
                        sp = dt_ == NT_D - 1
                        rhs = xt_t[:, half * 512:(half + 1) * 512]
                        for h in range(HPC):
                            nc.tensor.matmul(acc[h][:], wq_ap(dt_, h), rhs,
                                             start=st, stop=sp)
                        nc.tensor.matmul(acc[4][:], wk_ap(dt_), rhs,
                                         start=st, stop=sp)
                        nc.tensor.matmul(acc[5][:], wv_ap(dt_), rhs,
                                         start=st, stop=sp)
                for h in range(HPC):
                    rope_evac(acc[h], qt[:, h * S + ci * 512:h * S + (ci + 1) * 512], ci)
                rope_evac(acc[4], kt[:, ci * 512:(ci + 1) * 512], ci)
                # V: plain evac then PE-transpose (fp32, exact) to natural layout
                vt_t = tpool.tile([128, 512], FP32, tag="vt", bufs=2)
                nc.scalar.copy(vt_t[:], acc[5][:])
                for i in range(4):
                    ps_tr = psum.tile([128, 128], FP32, tag="tr", bufs=2,
                                      name=f"tr{ci}_{i}")
                    nc.tensor.transpose(ps_tr[:], vt_t[:, i * 128:(i + 1) * 128], id_sb[:])
                    s0 = (ci * 4 + i) * 128
                    nc.vector.tensor_copy(vn[:, s0:s0 + 128], ps_tr[:])

        if "B" not in phases:
            return

        # ================= Phase B: attention =================
        with ExitStack() as pb:
            e = pb.enter_context
            ppool = e(tc.tile_pool(name="ppool", bufs=4))
            npool = e(tc.tile_pool(name="npool", bufs=2))
            mpool = e(tc.tile_pool(name="mpool", bufs=4))
            psum = e(tc.tile_pool(name="psumB", bufs=1, space=bass.MemorySpace.PSUM))

            for ci in range(NCH):
                n_sk = 4 * (ci + 1) if mode == "causal" else S // 128
                for h in range(HPC):
                    ps_pv = psum.tile([128, 512], FP32, tag="pv", bufs=2,
                                      name=f"pv{ci}_{h}")
                    ps_sm = psum.tile([1, 512], FP32, tag="sm", bufs=2,
                                      name=f"sm{ci}_{h}")
                    qs = qt[:, h * S + ci * 512:h * S + (ci + 1) * 512]
                    for sk in range(n_sk):
                        ps_sc = psum.tile([128, 512], FP32, tag="sc", bufs=2,
                                          name=f"sc{ci}_{h}_{sk}")
                        nc.tensor.matmul(ps_sc[:], kt[:, sk * 128:(sk + 1) * 128],
                                         qs, start=True, stop=True)
                        p = ppool.tile([128, 512], MMDT, tag="p", bufs=4)
                        if mode == "masked":
                            mt = mpool.tile([128, 512], FP32, tag="mt", bufs=4)
                            nc.sync.dma_start(
                                mt[:], mskf_d[sk * 128:(sk + 1) * 128,
                                              ci * 512:(ci + 1) * 512])
                            nc.vector.tensor_scalar_mul(p[:], ps_sc[:], es)
                            nc.vector.tensor_add(p[:], p[:], mt[:])
                            nc.scalar.activation(p[:], p[:], AF.Exp,
                                                 bias=expb_c[:])
                        else:
                            nc.scalar.activation(p[:], ps_sc[:], AF.Exp,
                                                 scale=es, bias=expb_c[:])
                            if mode == "causal" and sk >= 4 * ci:
                                j = sk - 4 * ci
                                nc.vector.tensor_mul(
                                    p[:], p[:], msk_sb[:, j * 512:(j + 1) * 512])
                        st = sk == 0
                        sp = sk == n_sk - 1
                        nc.tensor.matmul(ps_pv[:], vn[:, sk * 128:(sk + 1) * 128],
                                         p[:], start=st, stop=sp)
                        nc.tensor.matmul(ps_sm[:], ones_c[:], p[:],
                                         start=st, stop=sp)
                    # normalize and fold in Wo row scales: outer product
                    # swo[h*128+p] * (1/sum[q]) via a K=1 matmul
                    rc = npool.tile([1, 512], FP32, tag="rc", bufs=2)
                    rrs = npool.tile([1, 512], FP32, tag="rs", bufs=2)
                    nc.vector.reciprocal_approx_accurate(rc[:], ps_sm[:], rrs[:])
                    ps_bc = psum.tile([128, 512], FP32, tag="bc", bufs=2,
                                      name=f"bc{ci}_{h}")
                    nc.tensor.matmul(ps_bc[:], swo_f[0:1, h * 128:(h + 1) * 128],
                                     rc[:], start=True, stop=True)
                    rb = npool.tile([128, 512], FP32, tag="rb", bufs=2)
                    nc.scalar.copy(rb[:], ps_bc[:])
                    nc.vector.tensor_mul(at[:, h * S + ci * 512:h * S + (ci + 1) * 512],
                                         ps_pv[:], rb[:])

        if DBG:
            nc.sync.dma_start(dbg["dqt"][:], qt[:])
            nc.sync.dma_start(dbg["dkt"][:], kt[:])
            nc.sync.dma_start(dbg["dvn"][:], vn[:])
            nc.sync.dma_start(dbg["dat"][:], at[:])
        if "C" not in phases:
            return
        # ================= Phase C: output projection -> DRAM partial =================
        with ExitStack() as pc:
            e = pc.enter_context
            wopool = e(tc.tile_pool(name="wopool", bufs=8))
            wo8pool = e(tc.tile_pool(name="wo8pool", bufs=3))
            opool = e(tc.tile_pool(name="opool", bufs=4))
            psum = e(tc.tile_pool(name="psumC", bufs=1, space=bass.MemorySpace.PSUM))
            wob = 1 if O8 else 2
            for op_ in range(D // 1024):
                wt = []
                for odh in range(2):
                    od = 2 * op_ + odh
                    w = wopool.tile([128, HPC * 512], MMDT, tag="wo", bufs=4)
                    off = OB_WO + od * HPC * 512 * wob
                    if O8:
                        w8 = wo8pool.tile([128, HPC * 512], INT8, tag="wo8", bufs=3)
                        nc.sync.dma_start(w8[:], blob_d[:, off:off + HPC * 512])
                        nc.scalar.copy(w[:], w8[:])
                    else:
                        nc.sync.dma_start(
                            w[:], blob_d[:, off:off + 2 * HPC * 512].bitcast(MMDT))
                    wt.append(w)
                for sb in range(S // 128):
                    ob = opool.tile([128, 1024], FP32, tag="ob", bufs=4)
                    for odh in range(2):
                        ps_o = psum.tile([128, 512], FP32, tag="oo", bufs=4,
                                         name=f"oo{op_}_{sb}_{odh}")
                        for h in range(HPC):
                            nc.tensor.matmul(
                                ps_o[:],
                                at[:, h * S + sb * 128:h * S + (sb + 1) * 128],
                                wt[odh][:, h * 512:(h + 1) * 512],
                                start=(h == 0), stop=(h == HPC - 1))
                        nc.vector.tensor_scalar_mul(ob[:, odh * 512:(odh + 1) * 512],
                                                    ps_o[:], 1.0 / CSC)
                    nc.sync.dma_start(po[sb * 128:(sb + 1) * 128,
                                         op_ * 1024:(op_ + 1) * 1024], ob[:])

        # ====== reduce-scatter partials across cores, emit fp16 shard ======
        with ExitStack() as pd_:
            e = pd_.enter_context
            spool = e(tc.tile_pool(name="spool", bufs=2))
            nc.gpsimd.collective_compute(
                "ReduceScatter", mybir.AluOpType.add, replica_groups=GROUPS,
                ins=[po.opt()], outs=[rs.opt()])
            for i in range(SROW // 128):
                t = spool.tile([128, D], FP32, tag="t", bufs=2)
                nc.sync.dma_start(t[:], rs[i * 128:(i + 1) * 128, :])
                tb = spool.tile([128, D], MMDT, tag="tb", bufs=2)
                nc.scalar.copy(tb[:], t[:])
                nc.sync.dma_start(out_d[i * 128:(i + 1) * 128, :], tb[:])


DBG = False   # add qt/kt/vn/at dump outputs (debug builds only)


def build(mode="causal", phases="ABC"):
    nc = bacc.Bacc("TRN2", target_bir_lowering=False, debug=False,
                   num_devices=NCORES)
    blob_d = nc.dram_tensor("blob", [128, NBYTES], INT8, kind="ExternalInput").ap()
    mskf_d = None
    if mode == "masked":
        mskf_d = nc.dram_tensor("msk", [S, S], FP32, kind="ExternalInput").ap()
    out_d = nc.dram_tensor("out", [SROW, D], MMDT, kind="ExternalOutput").ap()
    io = (blob_d, mskf_d, out_d)
    with tile.TileContext(nc) as tc:
        _emit(nc, tc, io, mode, phases)
    nc.compile()
    return nc


_CACHE = {}
RUN_KWARGS = {}   # extra kwargs for run_bass_kernel_spmd (e.g. trace=True)
LAST = None       # last BassKernelResults (for exec_time_ns inspection)


def _causal_ref_mask():
    neg = np.finfo(np.float32).min
    m = np.where(np.tril(np.ones((S, S), dtype=bool)), 0.0, neg)
    return m.astype(np.float32)


def _tile_rows(w):
    # [T*128, C] -> [128, T*C] with d-tile blocks along free dim
    t = w.shape[0] // 128
    return np.ascontiguousarray(
        w.reshape(t, 128, w.shape[1]).transpose(1, 0, 2).reshape(128, -1))


def _tile_wo(w):
    # [512, D] -> [128, (od, h) blocks]: block (h, od) at [p, od*2048 + h*512]
    return np.ascontiguousarray(
        w.reshape(HPC, 128, D // 512, 512).transpose(1, 2, 0, 3).reshape(128, -1))


def _quant8(w, r):
    # rowwise int8: round(w * r[:, None]) clipped to [-127, 127]
    y = w.astype(np.float32) * r[:, None]
    np.rint(y, out=y)
    np.clip(y, -127, 127, out=y)
    return y.astype(np.int8)


def make_in_maps(hidden_states, cos, sin, attention_mask, Wq, Wk, Wv, Wo, mode):
    mdt = _np_mmdt()
    Wq = np.asarray(Wq)
    Wk = np.asarray(Wk)
    Wv = np.asarray(Wv)
    Wo = np.asarray(Wo)
    # shared per-d-row scale for the int8 subset of Wq/Wk/Wv (folded into x);
    # unquantized ones get divided by the same row factor on the host.
    qsub = [w for w, f in ((Wq, Q8), (Wk, K8), (Wv, V8)) if f]
    if qsub:
        s6 = np.maximum.reduce([np.abs(w).max(1) for w in qsub])
        s6 = np.maximum(s6, 1e-30)
        u = (s6 / 127.0) * PSC                     # x row multiplier
        r6 = 127.0 / s6
    else:
        u = np.full(D, 1.0, np.float32)
        r6 = None
    bq = _quant8(Wq, r6) if Q8 else (Wq / u[:, None]).astype(mdt)
    bk = _quant8(Wk, r6) if K8 else (Wk / u[:, None]).astype(mdt)
    bv = _quant8(Wv, r6) if V8 else (Wv / u[:, None]).astype(mdt)
    pv_scale = PSC if V8 else 1.0
    if O8:
        so = np.maximum(np.abs(Wo).max(1), 1e-30)
        bo = _quant8(Wo, 127.0 / so)
        swo = (so / 127.0 * (CSC / pv_scale)).astype(mdt)     # [4096]
    else:
        bo = Wo.astype(mdt)
        swo = np.full(D, CSC / pv_scale, mdt)
    xT = np.asarray(hidden_states).reshape(S, D).T * u[:, None]
    xT = np.ascontiguousarray(xT).astype(mdt)                # [D, S] scaled
    cosT = np.ascontiguousarray(np.asarray(cos).T).astype(mdt)   # [128, S]
    sinT = np.ascontiguousarray(np.asarray(sin).T).astype(mdt)
    ident = np.eye(128, dtype=mdt)
    if mode == "causal":
        # 4 diagonal 0/1 tiles: tile j valid where 128*j + k <= q  (k:[128], q:[512])
        j = np.arange(4)[:, None, None]
        k = np.arange(128)[None, :, None]
        q = np.arange(512)[None, None, :]
        msk = np.ascontiguousarray((128 * j + k <= q).astype(mdt)
                                   .transpose(1, 0, 2).reshape(128, 2048))
    else:
        msk = np.zeros((128, 2048), dtype=mdt)
    mskf = None
    if mode == "masked":
        mskf = np.ascontiguousarray(
            np.asarray(attention_mask).reshape(S, S).T).astype(np.float32)

    def as8(a):
        return a.view(np.int8).reshape(a.shape[0], -1) if a.dtype != np.int8 else a

    in_maps = []
    for c in range(NCORES):
        blob = np.empty((128, NBYTES), dtype=np.int8)
        blob[:, OB_WQ:OB_WK] = as8(_tile_rows(bq[:, c * QC:(c + 1) * QC]))
        blob[:, OB_WK:OB_WV] = as8(_tile_rows(bk[:, c * DH:(c + 1) * DH]))
        blob[:, OB_WV:OB_WO] = as8(_tile_rows(bv[:, c * DH:(c + 1) * DH]))
        blob[:, OB_WO:OB_X] = as8(_tile_wo(bo[c * QC:(c + 1) * QC, :]))
        blob[:, OB_X:OB_CS] = xT[c * 512:(c + 1) * 512].view(np.int8).reshape(128, -1)
        blob[:, OB_CS:OB_SN] = cosT.view(np.int8)
        blob[:, OB_SN:OB_MSK] = sinT.view(np.int8)
        blob[:, OB_MSK:OB_ID] = msk.view(np.int8)
        blob[:, OB_ID:OB_SWO] = ident.view(np.int8)
        blob[:, OB_SWO:NBYTES] = 0
        blob[0, OB_SWO:NBYTES] = swo[c * QC:(c + 1) * QC].view(np.int8)
        m = {"blob": blob}
        if mode == "masked":
            m["msk"] = mskf
        in_maps.append(m)
    return in_maps


def pick_mode(attention_mask):
    am = np.asarray(attention_mask).reshape(S, S)
    if not np.any(am):
        return "dense"
    if np.array_equal(am, _causal_ref_mask()):
        return "causal"
    return "masked"


def kernel(hidden_states, cos, sin, attention_mask, Wq, Wk, Wv, Wo, **kwargs):
    mode = pick_mode(attention_mask)
    if mode not in _CACHE:
        _CACHE[mode] = build(mode)
    nc = _CACHE[mode]
    in_maps = make_in_maps(hidden_states, cos, sin, attention_mask,
                           Wq, Wk, Wv, Wo, mode)
    res = run_bass_kernel_spmd(nc, in_maps, core_ids=list(range(NCORES)),
                               **RUN_KWARGS)
    global LAST
    LAST = res
    out = np.concatenate([res.results[c]["out"] for c in range(NCORES)], axis=0)
    return out.astype(np.float32).reshape(1, S, D)
